# revision 1
# baseline (speedup 1.0000x reference)
"""Causal self-attention (B=4, T=2048, C=1024, H=16) on 8 TRN2 NeuronCores.

Sharding: data-parallel on batch (4) x tensor-parallel on heads (2 groups of
8). Core i handles batch i//2 and head-group i%2. Per core:
  - QKV matmuls for its head-group's weight columns. q,k are produced in
    transposed [feature, T] layout (lhsT = w chunk, rhs = x^T chunk); v in
    natural [T, feature] layout augmented with a ones column per head (the
    ones row of the attention output accumulates sum(exp)).
  - Causal attention per head in scores^T layout [k, q]. No max subtraction:
    scores*hs^-0.5 are O(+-10), exp is safe. Fully-masked k-blocks are
    skipped; diagonal blocks are zeroed after exp with a gpsimd
    affine_select.
  - y^T tiles are exchanged between the two cores of a batch with chunked
    pairwise AllGathers (one per head-pair) so comm overlaps attention.
  - Projection for the core's 512 output columns over the full sequence,
    accumulated in SBUF as AllGather chunks arrive. b_proj folded in.
Host slices/transposes inputs per core and concatenates the [2048, 512]
per-core outputs into the full [4, 2048, 1024] result.

dtypes: all matmul operands (x, w_qkv, q, k, p, v, y^T, w_proj) are bf16
(TensorE 1 cycle/row); every accumulation (scores, attention output, the
sum(exp) row, and the projection) is fp32 in PSUM, and softmax
normalization runs in fp32, so errors stay at the bf16-input level
(measured ~5e-3 fro vs the fp32 reference; gate is 2e-2).

Measured on HW: 466-472us exec, vs 679us for the first correct version.
PE-filler interleaving of QKV into the attention phase keeps the TensorE
dense (HAM clock-gate warm); AllGather results are consumed two
attention-pairs late so collective jitter never stalls the PE.
"""

import os
import sys
from contextlib import ExitStack

import numpy as np
import ml_dtypes

if "/opt/trn_rl_repo" not in sys.path:
    sys.path.insert(0, "/opt/trn_rl_repo")

import concourse.bass as bass
import concourse.mybir as mybir
import concourse.tile as tile
from concourse import bacc
from concourse import bass_utils

F32 = mybir.dt.float32
F32R = mybir.dt.float32r
BF16 = mybir.dt.bfloat16
P = 128          # SBUF partitions
QT = 512         # q tile (matmul free dim)
KC = 128         # k chunk (psum partition dim)
HS = 64          # head size
KPQ = QT // KC   # k chunks per q tile

N_CORES = 8
PAIRS = [[0, 1], [2, 3], [4, 5], [6, 7]]

# Full problem dims (hardcoded; kernel.py must be self-contained).
B_FULL, T_FULL, C_FULL, H_FULL = 4, 2048, 1024, 16


def build_nc(T=T_FULL, C=C_FULL, HL=H_FULL // 2):
    """Build the SPMD graph for one core (all 8 cores run the same graph).

    Per-core input tensors (names = in_maps keys):
      xT    [C, T]  bf16   x[b] transposed
      wqkT  [C, 2*CL] bf16 w_attn columns for local q then local k
      wv    [C, CL] bf16   w_attn columns for local v
      bqk   [2*CL] f32
      bv    [CL] f32
      wp    [C, CL] f32r   w_proj columns for this core's output half
      bp    [CL] f32
    Output: out [T, CL] f32.
    """
    CL = HL * HS                 # local width (q, k, v, and out-cols each)
    n_cc = C // P                # x feature chunks
    n_fqk = 2 * CL // P          # q|k f-tiles (first half q, second half k)
    n_jt = T // QT               # q tiles
    n_kt = T // KC               # k chunks (also v t-chunks)
    n_pair = HL // 2             # head pairs (= AG chunk count)
    scale = HS ** -0.5
    assert n_cc == 2 * n_pair

    nc = bacc.Bacc("TRN2", target_bir_lowering=False, debug=False,
                   num_devices=N_CORES)

    xT = nc.dram_tensor("xT", [C, T], BF16, kind="ExternalInput").ap()
    wqkT = nc.dram_tensor("wqkT", [C, 2 * CL], BF16, kind="ExternalInput").ap()
    wv = nc.dram_tensor("wv", [C, CL], BF16, kind="ExternalInput").ap()
    bqk = nc.dram_tensor("bqk", [2 * CL], F32, kind="ExternalInput").ap()
    bv = nc.dram_tensor("bv", [CL], F32, kind="ExternalInput").ap()
    wp = nc.dram_tensor("wp", [C, CL], BF16, kind="ExternalInput").ap()
    bp = nc.dram_tensor("bp", [CL], F32, kind="ExternalInput").ap()
    out_ext = nc.dram_tensor("out", [T, CL], F32, kind="ExternalOutput").ap()

    with ExitStack() as ctx:
        tc = ctx.enter_context(tile.TileContext(nc))

        persist = ctx.enter_context(tc.tile_pool(name="persist", bufs=1))
        dram = ctx.enter_context(tc.tile_pool(name="dram", bufs=1, space="DRAM"))
        # one PSUM pool for the whole kernel: qps 2 + st 2x2 + yp 2 = 8 banks
        ps = ctx.enter_context(tc.tile_pool(name="ps", bufs=1, space="PSUM"))

        # ---- constants -------------------------------------------------
        bqk_sb = persist.tile([P, n_fqk], F32, tag="bqk", name="bqk_sb")
        nc.sync.dma_start(bqk_sb[:], bqk.rearrange("(f p) -> p f", p=P))
        bv_bc = persist.tile([P, CL], F32, tag="bv_bc", name="bv_bc")
        bp_bc = persist.tile([P, CL], F32, tag="bp_bc", name="bp_bc")
        ones_f = persist.tile([P, HL, 1], F32, tag="ones_f", name="ones_f")
        nc.gpsimd.memset(ones_f[:], 1.0)

        # ---- stage 1: QKV ---------------------------------------------
        qk_sb = [persist.tile([P, T], BF16, tag=f"qk{f}", name=f"qk{f}")
                 for f in range(n_fqk)]
        v_sb = [persist.tile([P, HL, HS + 2], BF16, tag=f"v{t}", name=f"v{t}")
                for t in range(n_kt)]

        xpool = ctx.enter_context(tc.tile_pool(name="xpool", bufs=1))
        wpool = ctx.enter_context(tc.tile_pool(name="wpool", bufs=3))

        wv_sb = []
        for c in range(n_cc):
            wvt = wpool.tile([P, CL], BF16, tag=f"wv{c}", bufs=1,
                             name=f"wv{c}")
            nc.sync.dma_start(wvt[:], wv[c * P:(c + 1) * P, :])
            wv_sb.append(wvt)
        x_sb = []
        for c in range(n_cc):
            xt = xpool.tile([P, T], BF16, tag=f"x{c}", name=f"x{c}")
            eng = nc.sync if c % 2 == 0 else nc.gpsimd
            eng.dma_start(xt[:], xT[c * P:(c + 1) * P, :])
            x_sb.append(xt)

        bv_row = wpool.tile([1, CL], F32, tag="bv_row", bufs=1,
                            name="bv_row")
        nc.sync.dma_start(bv_row[:], bv.rearrange("(o c) -> o c", o=1))
        nc.gpsimd.partition_broadcast(bv_bc[:], bv_row[:])
        bp_row = wpool.tile([1, CL], F32, tag="bp_row", bufs=1,
                            name="bp_row")
        nc.sync.dma_start(bp_row[:], bp.rearrange("(o c) -> o c", o=1))
        nc.gpsimd.partition_broadcast(bp_bc[:], bp_row[:])

        # V in natural layout: [t 128, CL] tiles, ones column per head
        for t in range(n_kt):
            pv = ps.tile([P, CL], F32, tag="qps", bufs=2, name="pv")
            for c in range(n_cc):
                nc.tensor.matmul(
                    pv[:], x_sb[c][:, t * KC:(t + 1) * KC], wv_sb[c][:],
                    start=(c == 0), stop=(c == n_cc - 1))
            nc.vector.tensor_copy(v_sb[t][:, :, HS:HS + 1], ones_f[:])
            nc.vector.tensor_add(
                v_sb[t][:, :, 0:HS],
                pv.rearrange("p (h e) -> p h e", e=HS),
                bv_bc.rearrange("p (h e) -> p h e", e=HS))

        def qk_unit(f, th):
            """One q/k f-tile x t-half: 16 matmuls + bias copies to SBUF."""
            us = [u for u in range(2) if 2 * th + u < n_jt]
            pts = {u: ps.tile([P, QT], F32, tag="qps", bufs=2,
                              name=f"pqk{u}") for u in us}
            for c in range(n_cc):
                wt = wpool.tile([P, P], BF16, tag="wqk", name="wqk")
                nc.sync.dma_start(wt[:], wqkT[c * P:(c + 1) * P,
                                              f * P:(f + 1) * P])
                for u in us:
                    t = 2 * th + u
                    nc.tensor.matmul(
                        pts[u][:], wt[:],
                        x_sb[c][:, t * QT:(t + 1) * QT],
                        start=(c == 0), stop=(c == n_cc - 1))
            for u in us:
                t = 2 * th + u
                nc.vector.tensor_scalar_add(
                    qk_sb[f][:, t * QT:(t + 1) * QT], pts[u][:],
                    bqk_sb[:, f:f + 1])

        n_th = (n_jt + 1) // 2
        # pair-0 q/k upfront; later pairs are emitted as PE fillers between
        # attention sections so the TensorE stays dense (and HAM-warm)
        for f in (0, n_pair):
            for th in range(n_th):
                qk_unit(f, th)
        filler = []
        for pr in range(1, n_pair):
            for f in (pr, n_pair + pr):
                for th in range(n_th):
                    filler.append((f, th))
        filler.reverse()  # pop() from the front of the logical order

        # ---- stage 2+3: attention + chunked AllGather + projection ----
        ag_in = [dram.tile([P, T], BF16, tag=f"agin{p}", name=f"agin{p}")
                 for p in range(n_pair)]
        ag_out = [dram.tile([2, P, T], BF16, tag=f"agout{p}", name=f"agout{p}")
                  for p in range(n_pair)]

        att = ctx.enter_context(tc.tile_pool(name="att", bufs=1))

        wp_sb = [att.tile([P, CL], BF16, tag=f"wp{c}", name=f"wp{c}")
                 for c in range(n_cc)]
        for c in range(n_cc):
            nc.sync.dma_start(wp_sb[c][:], wp[c * P:(c + 1) * P, :])
        oacc = [att.tile([P, CL], F32, tag=f"oacc{t}", name=f"oacc{t}")
                for t in range(T // P)]

        def attention_pair(pr):
            """Both heads of pair pr jointly: each stripe holds head A's and
            head B's scores side by side, so one exp covers both and the PE
            runs 4 matmuls per ACT call."""
            kT = qk_sb[n_pair + pr]
            qTt = qk_sb[pr]
            for j in range(n_jt):
                yps = {rr: ps.tile([P, QT], F32, tag=f"yp{rr}", bufs=1,
                                   name=f"yp{rr}") for rr in range(2)}
                imax = KPQ * j + KPQ
                for i in range(imax):
                    st = ps.tile([P, 2 * QT], F32, tag="st", bufs=2,
                                 name="st")
                    for rr in range(2):
                        ro = HS * rr
                        nc.tensor.matmul(
                            st[:, rr * QT:(rr + 1) * QT],
                            kT[ro:ro + HS, i * KC:(i + 1) * KC],
                            qTt[ro:ro + HS, j * QT:(j + 1) * QT],
                            start=True, stop=True)
                    pt = att.tile([P, 2 * QT], BF16, tag="pt", bufs=4,
                                  name="pt")
                    nc.scalar.activation(
                        pt[:], st[:],
                        mybir.ActivationFunctionType.Exp, scale=scale)
                    if i // KPQ == j:
                        for rr in range(2):
                            # zero above the causal diagonal:
                            # keep where col f >= p + 128*(i % KPQ)
                            nc.gpsimd.affine_select(
                                out=pt[:, rr * QT:(rr + 1) * QT],
                                in_=pt[:, rr * QT:(rr + 1) * QT],
                                compare_op=mybir.AluOpType.is_ge,
                                fill=0.0, base=-KC * (i % KPQ),
                                channel_multiplier=-1, pattern=[[1, QT]])
                    for rr in range(2):
                        nc.tensor.matmul(
                            yps[rr][0:HS + 1, :],
                            v_sb[i][:, 2 * pr + rr, 0:HS + 1],
                            pt[:, rr * QT:(rr + 1) * QT],
                            start=(i == 0), stop=(i == imax - 1))
                for rr in range(2):
                    ro = HS * rr
                    # custom-DVE recip misreads PSUM: stage the sum row in
                    # SBUF first (cheap), then approx-reciprocal there
                    row = att.tile([1, QT], F32, tag="row", bufs=3,
                                   name="row")
                    nc.vector.tensor_copy(row[:], yps[rr][HS:HS + 1, :])
                    rec = att.tile([1, QT], F32, tag="rec", bufs=3,
                                   name="rec")
                    nc.vector.reciprocal_approx_fast(rec[:], row[:])
                    rb = att.tile([HS, QT], F32, tag="rb", bufs=3, name="rb")
                    nc.gpsimd.partition_broadcast(rb[:], rec[:])
                    yn = att.tile([HS, QT], BF16, tag="yn", bufs=4,
                                  name="yn")
                    nc.vector.tensor_mul(yn[:], yps[rr][0:HS, :], rb[:])
                    nc.sync.dma_start(
                        ag_in[pr][ro:ro + HS, j * QT:(j + 1) * QT], yn[:])
                if filler:
                    qk_unit(*filler.pop())

        def proj_chunk(p, final=False):
            # consume AG chunk p: global c-chunks gp*n_pair + p, gp in (0,1)
            for t in range(T // P):
                po = ps.tile([P, CL], F32, tag="qps", bufs=2, name="po")
                yts = []
                for gp in range(2):
                    yt = att.tile([P, P], BF16, tag="yt", bufs=6, name="yt")
                    nc.sync.dma_start(yt[:],
                                      ag_out[p][gp, :, t * P:(t + 1) * P])
                    yts.append(yt)
                for gp in range(2):
                    nc.tensor.matmul(po[:], yts[gp][:],
                                     wp_sb[gp * n_pair + p][:],
                                     start=(gp == 0), stop=(gp == 1))
                if p == 0:
                    nc.vector.tensor_add(oacc[t][:], po[:], bp_bc[:])
                else:
                    nc.vector.tensor_add(oacc[t][:], oacc[t][:], po[:])
                if final:
                    nc.sync.dma_start(out_ext[t * P:(t + 1) * P, :],
                                      oacc[t][:])

        for pr in range(n_pair):
            attention_pair(pr)
            nc.gpsimd.collective_compute(
                "AllGather", mybir.AluOpType.bypass,
                replica_groups=PAIRS,
                ins=[ag_in[pr].opt()], outs=[ag_out[pr].opt()])
            # consume AG chunks 1-2 attention-pairs after issue (measured
            # AG completion is 4-17us; a pair of heads is ~65us) so only the
            # last chunk's projection remains in the tail
            if n_pair >= 4:
                if pr == 2:
                    proj_chunk(0)
                    proj_chunk(1)
                elif pr == 3:
                    proj_chunk(2)
        done = [0, 1, 2] if n_pair >= 4 else []
        rest = [p for p in range(n_pair) if p not in done]
        for p in rest:
            proj_chunk(p, final=(p == rest[-1]))


    nc.compile()
    return nc


def shard_inputs(x, w_attn, b_attn, w_proj, b_proj):
    """Slice/transpose full inputs into 8 per-core input maps."""
    Bq, T, C = x.shape
    CL = C // 2
    bf = ml_dtypes.bfloat16
    in_maps = []
    for i in range(N_CORES):
        b, g = i // 2, i % 2
        sl = slice(CL * g, CL * (g + 1))
        wq = w_attn[:, sl]
        wk = w_attn[:, C + CL * g:C + CL * (g + 1)]
        wvv = w_attn[:, 2 * C + CL * g:2 * C + CL * (g + 1)]
        in_maps.append({
            "xT": np.ascontiguousarray(x[b].T).astype(bf),
            "wqkT": np.ascontiguousarray(
                np.concatenate([wq, wk], axis=1)).astype(bf),
            "wv": np.ascontiguousarray(wvv).astype(bf),
            "bqk": np.ascontiguousarray(
                np.concatenate([b_attn[sl],
                                b_attn[C + CL * g:C + CL * (g + 1)]])),
            "bv": np.ascontiguousarray(b_attn[2 * C + CL * g:2 * C + CL * (g + 1)]),
            "wp": np.ascontiguousarray(w_proj[:, sl]).astype(bf),
            "bp": np.ascontiguousarray(b_proj[sl]),
        })
    return in_maps


def gather_outputs(results, B, T, C):
    CL = C // 2
    out = np.empty((B, T, C), dtype=np.float32)
    for i in range(N_CORES):
        b, g = i // 2, i % 2
        out[b, :, CL * g:CL * (g + 1)] = results[i]["out"]
    return out


_NC_CACHE = {}


def get_nc(T, C):
    key = (T, C)
    if key not in _NC_CACHE:
        _NC_CACHE[key] = build_nc(T=T, C=C, HL=C // HS // 2)
    return _NC_CACHE[key]


def kernel(x, w_attn, b_attn, w_proj, b_proj):
    x = np.asarray(x, dtype=np.float32)
    w_attn = np.asarray(w_attn, dtype=np.float32)
    b_attn = np.asarray(b_attn, dtype=np.float32)
    w_proj = np.asarray(w_proj, dtype=np.float32)
    b_proj = np.asarray(b_proj, dtype=np.float32)

    Bq, T, C = x.shape
    nc = get_nc(T, C)

    in_maps = shard_inputs(x, w_attn, b_attn, w_proj, b_proj)
    trace = os.environ.get("KERNEL_TRACE", "0") == "1"
    res = bass_utils.run_bass_kernel_spmd(
        nc, in_maps, core_ids=list(range(N_CORES)), trace=trace)
    if trace and res.exec_time_ns is not None:
        print(f"HW exec time: {res.exec_time_ns} ns", flush=True)
        kernel.last_exec_time_ns = res.exec_time_ns
        kernel.last_results = res
    return gather_outputs(res.results, Bq, T, C)



# revision 5
# speedup vs baseline: 1.1614x; 1.1614x over previous
"""Causal self-attention (B=4, T=2048, C=1024, H=16) on 8 TRN2 NeuronCores.

Sharding: data-parallel on batch (4) x tensor-parallel on heads (2 groups of
8). Core i handles batch i//2 and head-group i%2. Per core:
  - QKV matmuls for its head-group's weight columns. q,k are produced in
    transposed [feature, T] layout; v in natural [T, feature] layout with a
    ones column per head (sum(exp) accumulates in the attention matmul).
  - Causal attention per head-pair in scores^T layout [k, q]. No max
    subtraction: scores*hs^-0.5 are O(+-10), exp is safe. Fully-masked
    k-blocks are skipped; diagonal blocks are N-trimmed to the causal q-range
    and the remaining 128x128 triangle is masked with a DVE multiply against
    a precomputed 0/1 mask (NOT gpsimd affine_select - that serialized the
    collective queue and stalled the tail at half HAM clock).
  - The score->exp->att.v chain is software-pipelined 2 deep: PE emits
    st(i), ACT exp(i), PE av(i-2), so the PE never waits on the ACT engine.
    Filler work (V chunks, later pairs' q/k tiles, projection tiles) is
    injected every other iteration to keep the PE dense (HAM clock warm).
  - y^T is exchanged between the two cores of a batch with per-(pair,T-half)
    pairwise AllGathers (8 small collectives) so only the last T-half's
    projection remains in the tail.
  - Projection accumulates in SBUF as AG chunks arrive; b_proj folded in.
Host shuffles weights into [p, c, m]-style layouts so every weight DMA moves
2KB+ contiguous lines per partition; x streams in T-half-major order so the
first matmuls start after ~2.5MB instead of the full 8MB.

dtypes: all matmul operands bf16; every accumulation fp32 in PSUM; softmax
normalization fp32 (measured ~5e-3 fro vs fp32 reference; gate is 2e-2).
"""

import os
import sys
from contextlib import ExitStack

import numpy as np
import ml_dtypes

if "/opt/trn_rl_repo" not in sys.path:
    sys.path.insert(0, "/opt/trn_rl_repo")

import concourse.bass as bass
import concourse.mybir as mybir
import concourse.tile as tile
from concourse import bacc
from concourse import bass_utils

F32 = mybir.dt.float32
BF16 = mybir.dt.bfloat16
P = 128          # SBUF partitions
QT = 512         # q tile (matmul free dim)
KC = 128         # k chunk (psum partition dim)
HS = 64          # head size
KPQ = QT // KC   # k chunks per q tile

N_CORES = 8
PAIRS = [[0, 1], [2, 3], [4, 5], [6, 7]]

B_FULL, T_FULL, C_FULL, H_FULL = 4, 2048, 1024, 16


def build_nc(T=T_FULL, C=C_FULL, HL=H_FULL // 2):
    """Build the SPMD graph for one core (all 8 cores run the same graph).

    Per-core input tensors:
      xT    [C, T] bf16       x[b] transposed
      wqk   [2CL/P, P, C/P, P] bf16  w_attn q|k cols, host-shuffled [f,p,c,m]
      wv_s  [P, C/P, CL] bf16 w_attn v cols, host-shuffled [p,c,m]
      wp_s  [P, C/P, CL] bf16 w_proj cols for this core's output half
      bqk   [2*CL] f32, bv [CL] f32, bp [CL] f32
    Output: out [T, CL] f32.
    """
    CL = HL * HS                 # local width (q, k, v, out-cols each)
    n_cc = C // P                # x feature chunks (8)
    n_f = 2 * CL // P            # q|k f-tiles (4 q then 4 k)
    n_jt = T // QT               # q tiles (4)
    n_kt = T // KC               # k chunks / v t-chunks (16)
    n_pair = HL // 2             # head pairs (4)
    n_half = 2                   # T halves for AG chunking
    TH = T // n_half
    scale = HS ** -0.5

    nc = bacc.Bacc("TRN2", target_bir_lowering=False, debug=False,
                   num_devices=N_CORES)

    xT = nc.dram_tensor("xT", [C, T], BF16, kind="ExternalInput").ap()
    wqk = nc.dram_tensor("wqk", [n_f, P, n_cc, P], BF16,
                         kind="ExternalInput").ap()
    wv_s = nc.dram_tensor("wv_s", [P, n_cc, CL], BF16,
                          kind="ExternalInput").ap()
    wp_s = nc.dram_tensor("wp_s", [P, n_cc, CL], BF16,
                          kind="ExternalInput").ap()
    bqk = nc.dram_tensor("bqk", [2 * CL], F32, kind="ExternalInput").ap()
    bv = nc.dram_tensor("bv", [CL], F32, kind="ExternalInput").ap()
    bp = nc.dram_tensor("bp", [CL], F32, kind="ExternalInput").ap()
    out_ext = nc.dram_tensor("out", [T, CL], F32, kind="ExternalOutput").ap()

    with ExitStack() as ctx:
        tc = ctx.enter_context(tile.TileContext(nc))

        persist = ctx.enter_context(tc.tile_pool(name="persist", bufs=1))
        dram = ctx.enter_context(tc.tile_pool(name="dram", bufs=1, space="DRAM"))
        # st 2x2 banks + yp0 + yp1 + qps 2 = 8 banks
        ps = ctx.enter_context(tc.tile_pool(name="ps", bufs=1, space="PSUM"))
        att = ctx.enter_context(tc.tile_pool(name="att", bufs=1))

        # ---- persistent SBUF tiles -----------------------------------
        wqk_sb = [persist.tile([P, n_cc, P], BF16, tag=f"wqk{f}",
                               name=f"wqk{f}") for f in range(n_f)]
        wv_sb = persist.tile([P, n_cc, CL], BF16, tag="wv", name="wv")
        wp_sb = persist.tile([P, n_cc, CL], BF16, tag="wp", name="wp")
        x_sb = [persist.tile([P, T], BF16, tag=f"x{c}", name=f"x{c}")
                for c in range(n_cc)]
        qk_sb = [persist.tile([P, T], BF16, tag=f"qk{f}", name=f"qk{f}")
                 for f in range(n_f)]
        v_sb = [persist.tile([P, HL, HS + 2], BF16, tag=f"v{t}",
                             name=f"v{t}") for t in range(n_kt)]
        oacc = [persist.tile([P, CL], F32, tag=f"oacc{t}", name=f"oacc{t}")
                for t in range(T // P)]
        bqk_sb = persist.tile([P, n_f], F32, tag="bqk", name="bqk_sb")
        bv_bc = persist.tile([P, CL], F32, tag="bv_bc", name="bv_bc")
        bp_bc = persist.tile([P, CL], F32, tag="bp_bc", name="bp_bc")
        ones_f = persist.tile([P, HL, 1], F32, tag="ones_f", name="ones_f")
        # diag_mask[u][p, q] = 1 where q >= p + 128*u else 0 (full q width)
        diag_mask = [persist.tile([P, QT], BF16, tag=f"dm{u}", name=f"dm{u}")
                     for u in range(KPQ)]

        # ---- input DMAs, in consumption order ------------------------
        # first PE work needs wqk f=0,4 + x half 0; then wv; then x half 1
        nc.sync.dma_start(wqk_sb[0][:], wqk[0])
        nc.sync.dma_start(wqk_sb[n_jt][:], wqk[n_jt])
        for h in range(2):
            for c in range(n_cc):
                eng = nc.sync if c % 2 == 0 else nc.gpsimd
                eng.dma_start(x_sb[c][:, h * TH:(h + 1) * TH],
                              xT[c * P:(c + 1) * P, h * TH:(h + 1) * TH])
            if h == 0:
                nc.sync.dma_start(wv_sb[:], wv_s)
        for f in range(n_f):
            if f not in (0, n_jt):
                nc.sync.dma_start(wqk_sb[f][:], wqk[f])
        nc.sync.dma_start(wp_sb[:], wp_s)
        nc.sync.dma_start(bqk_sb[:], bqk.rearrange("(f p) -> p f", p=P))

        bv_row = att.tile([1, CL], F32, tag="brow", bufs=2, name="bv_row")
        nc.sync.dma_start(bv_row[:], bv.rearrange("(o c) -> o c", o=1))
        nc.gpsimd.partition_broadcast(bv_bc[:], bv_row[:])
        bp_row = att.tile([1, CL], F32, tag="brow", bufs=2, name="bp_row")
        nc.sync.dma_start(bp_row[:], bp.rearrange("(o c) -> o c", o=1))
        nc.gpsimd.partition_broadcast(bp_bc[:], bp_row[:])
        nc.gpsimd.memset(ones_f[:], 1.0)
        for u in range(KPQ):
            nc.gpsimd.memset(diag_mask[u][:], 1.0)
            nc.gpsimd.affine_select(
                out=diag_mask[u][:], in_=diag_mask[u][:],
                compare_op=mybir.AluOpType.is_ge,
                fill=0.0, base=-KC * u, channel_multiplier=-1,
                pattern=[[1, QT]])

        # ---- AG buffers: one per (pair, T-half) ----------------------
        ag_in = [[dram.tile([P, TH], BF16, tag=f"agi{p}_{h}",
                            name=f"agi{p}_{h}") for h in range(n_half)]
                 for p in range(n_pair)]
        ag_out = [[dram.tile([2, P, TH], BF16, tag=f"ago{p}_{h}",
                             name=f"ago{p}_{h}") for h in range(n_half)]
                  for p in range(n_pair)]

        # ---- compute atoms -------------------------------------------
        def v_atom(t):
            """V for t-chunk t: [128 t, CL] + bias, ones col per head."""
            pv = ps.tile([P, CL], F32, tag="qps", bufs=2, name="pv")
            for c in range(n_cc):
                nc.tensor.matmul(
                    pv[:], x_sb[c][:, t * KC:(t + 1) * KC], wv_sb[:, c, :],
                    start=(c == 0), stop=(c == n_cc - 1))
            nc.vector.tensor_copy(v_sb[t][:, :, HS:HS + 1], ones_f[:])
            nc.vector.tensor_add(
                v_sb[t][:, :, 0:HS],
                pv.rearrange("p (h e) -> p h e", e=HS),
                bv_bc.rearrange("p (h e) -> p h e", e=HS))

        def qk_atom(f, t):
            """q/k f-tile x one t-tile of 512: 8 matmuls + bias to SBUF."""
            pq = ps.tile([P, QT], F32, tag="qps", bufs=2, name="pq")
            for c in range(n_cc):
                nc.tensor.matmul(
                    pq[:], wqk_sb[f][:, c, :],
                    x_sb[c][:, t * QT:(t + 1) * QT],
                    start=(c == 0), stop=(c == n_cc - 1))
            nc.vector.tensor_scalar_add(
                qk_sb[f][:, t * QT:(t + 1) * QT], pq[:], bqk_sb[:, f:f + 1])

        proj_pend = []   # prefetched (p, t, [yt0, yt1]) awaiting matmul

        def proj_fetch(p, t):
            yts = []
            for gp in range(2):
                yt = att.tile([P, P], BF16, tag="yt", bufs=8, name="yt")
                nc.sync.dma_start(
                    yt[:], ag_out[p][t // (TH // P)][gp, :,
                                                     (t * P) % TH:
                                                     (t * P) % TH + P])
                yts.append(yt)
            proj_pend.append((p, t, yts))

        def proj_exec(p, t, yts):
            po = ps.tile([P, CL], F32, tag="qps", bufs=2, name="po")
            for gp in range(2):
                nc.tensor.matmul(po[:], yts[gp][:],
                                 wp_sb[:, gp * n_pair + p, :],
                                 start=(gp == 0), stop=(gp == 1))
            if p == 0:
                nc.vector.tensor_add(oacc[t][:], po[:], bp_bc[:])
            else:
                nc.vector.tensor_add(oacc[t][:], oacc[t][:], po[:])
            if p == n_pair - 1:
                nc.sync.dma_start(out_ext[t * P:(t + 1) * P, :], oacc[t][:])

        def proj_atom(p, t):
            """Projection tile with 2-deep DMA prefetch."""
            proj_fetch(p, t)
            if len(proj_pend) > 2:
                proj_exec(*proj_pend.pop(0))

        def proj_drain():
            while proj_pend:
                proj_exec(*proj_pend.pop(0))

        # ---- filler queue --------------------------------------------
        # (min_pair, min_j, thunk): atom may only be emitted at or after
        # attention position (min_pair, min_j) - proj needs its AG landed.
        filler = []
        for t in range(8, n_kt):
            filler.append((0, 0, (lambda t=t: v_atom(t))))
        for pr in range(1, n_pair):
            for t in range(n_jt):
                filler.append((0, 0, (lambda f=n_jt + pr, t=t: qk_atom(f, t))))
                filler.append((0, 0, (lambda f=pr, t=t: qk_atom(f, t))))
        for p in range(n_pair):
            for t in range(T // P):
                # AG half h of pair p is issued after (p, 2h+1); give it a
                # pair of j-slots to land before projection consumes it.
                h = t // (TH // P)
                mp, mj = p + (1 if h == 0 else 2), (3 if h == 0 else 1)
                if mp > n_pair - 1:
                    mp, mj = n_pair - 1, n_jt  # drains in the tail
                filler.append((mp, mj, (lambda p=p, t=t: proj_atom(p, t))))

        def pop_filler(pr, j):
            for idx, (mp, mj, thunk) in enumerate(filler):
                if (mp, mj) <= (pr, j):
                    filler.pop(idx)
                    thunk()
                    return True
            return False

        # ---- attention -----------------------------------------------
        def attention_pair(pr):
            """Both heads of pair pr; scores^T [k, q] stripes, 2-deep
            pipelined st -> exp -> av so PE never waits on ACT."""
            kT = qk_sb[n_pair + pr]
            qTt = qk_sb[pr]
            for j in range(n_jt):
                yps = {rr: ps.tile([P, QT], F32, tag=f"yp{rr}", bufs=1,
                                   name=f"yp{rr}") for rr in range(2)}
                imax = KPQ * j + KPQ
                pend = []   # pipelined (i, pt) awaiting av
                for i in range(imax):
                    diag = (i // KPQ == j)
                    st = ps.tile([P, 2, QT], F32, tag="st", bufs=2,
                                 name="st")
                    for rr in range(2):
                        ro = HS * rr
                        nc.tensor.matmul(
                            st[:, rr, :],
                            kT[ro:ro + HS, i * KC:(i + 1) * KC],
                            qTt[ro:ro + HS, j * QT:(j + 1) * QT],
                            start=True, stop=True)
                    pt = att.tile([P, 2, QT], BF16, tag="pt", bufs=4,
                                  name="pt")
                    nc.scalar.activation(
                        pt[:], st[:],
                        mybir.ActivationFunctionType.Exp, scale=scale)
                    if diag:
                        u = i % KPQ
                        for rr in range(2):
                            # zero above the causal diagonal in place
                            nc.vector.tensor_mul(
                                pt[:, rr, :], pt[:, rr, :],
                                diag_mask[u][:])
                    pend.append((i, pt))
                    if len(pend) > 2:
                        iv, ptv = pend.pop(0)
                        for rr in range(2):
                            nc.tensor.matmul(
                                yps[rr][0:HS + 1, :],
                                v_sb[iv][:, 2 * pr + rr, 0:HS + 1],
                                ptv[:, rr, :],
                                start=(iv == 0), stop=(iv == imax - 1))
                    if i % 2 == 1:
                        pop_filler(pr, j)
                while pend:
                    iv, ptv = pend.pop(0)
                    for rr in range(2):
                        nc.tensor.matmul(
                            yps[rr][0:HS + 1, :],
                            v_sb[iv][:, 2 * pr + rr, 0:HS + 1],
                            ptv[:, rr, :],
                            start=(iv == 0), stop=(iv == imax - 1))
                for rr in range(2):
                    ro = HS * rr
                    # custom-DVE recip misreads PSUM: stage sum row in SBUF
                    row = att.tile([1, QT], F32, tag="row", bufs=3,
                                   name="row")
                    nc.vector.tensor_copy(row[:], yps[rr][HS:HS + 1, :])
                    rec = att.tile([1, QT], F32, tag="rec", bufs=3,
                                   name="rec")
                    nc.vector.reciprocal_approx_fast(rec[:], row[:])
                    rb = att.tile([HS, QT], F32, tag="rb", bufs=3, name="rb")
                    nc.gpsimd.partition_broadcast(rb[:], rec[:])
                    yn = att.tile([HS, QT], BF16, tag="yn", bufs=4,
                                  name="yn")
                    nc.vector.tensor_mul(yn[:], yps[rr][0:HS, :], rb[:])
                    h = j // 2
                    nc.sync.dma_start(
                        ag_in[pr][h][ro:ro + HS,
                                     (j % 2) * QT:(j % 2 + 1) * QT], yn[:])
                if j % 2 == 1:
                    h = j // 2
                    nc.gpsimd.collective_compute(
                        "AllGather", mybir.AluOpType.bypass,
                        replica_groups=PAIRS,
                        ins=[ag_in[pr][h].opt()], outs=[ag_out[pr][h].opt()])

        # ---- schedule ------------------------------------------------
        # upfront: pair-0 q/k and V for the first half of the sequence
        for t in range(2):
            qk_atom(0, t)
            qk_atom(n_jt, t)
        for t in range(8):
            v_atom(t)
        for t in range(2, n_jt):
            qk_atom(0, t)
            qk_atom(n_jt, t)

        for pr in range(n_pair):
            attention_pair(pr)
        # drain remaining fillers (last AG half's projection tiles)
        while pop_filler(n_pair - 1, n_jt):
            pass
        proj_drain()

    nc.compile()
    return nc


def shard_inputs(x, w_attn, b_attn, w_proj, b_proj):
    """Slice/transpose/shuffle full inputs into 8 per-core input maps."""
    Bq, T, C = x.shape
    CL = C // 2
    n_cc = C // P
    n_f = 2 * CL // P
    bf = ml_dtypes.bfloat16
    in_maps = []
    for i in range(N_CORES):
        b, g = i // 2, i % 2
        sl = slice(CL * g, CL * (g + 1))
        wq = w_attn[:, sl]
        wk = w_attn[:, C + CL * g:C + CL * (g + 1)]
        wvv = w_attn[:, 2 * C + CL * g:2 * C + CL * (g + 1)]
        wqk = np.concatenate([wq, wk], axis=1)          # [C, 2CL]
        # [C, 2CL] -> [f, p, c, m]: row r = c*128+p, col = f*128+m
        wqk_s = np.ascontiguousarray(
            wqk.reshape(n_cc, P, n_f, P).transpose(2, 1, 0, 3)).astype(bf)
        wv_shuf = np.ascontiguousarray(
            wvv.reshape(n_cc, P, CL).transpose(1, 0, 2)).astype(bf)
        wp_shuf = np.ascontiguousarray(
            w_proj[:, sl].reshape(n_cc, P, CL).transpose(1, 0, 2)).astype(bf)
        in_maps.append({
            "xT": np.ascontiguousarray(x[b].T).astype(bf),
            "wqk": wqk_s,
            "wv_s": wv_shuf,
            "wp_s": wp_shuf,
            "bqk": np.ascontiguousarray(
                np.concatenate([b_attn[sl],
                                b_attn[C + CL * g:C + CL * (g + 1)]])),
            "bv": np.ascontiguousarray(b_attn[2 * C + CL * g:2 * C + CL * (g + 1)]),
            "bp": np.ascontiguousarray(b_proj[sl]),
        })
    return in_maps


def gather_outputs(results, B, T, C):
    CL = C // 2
    out = np.empty((B, T, C), dtype=np.float32)
    for i in range(N_CORES):
        b, g = i // 2, i % 2
        out[b, :, CL * g:CL * (g + 1)] = results[i]["out"]
    return out


_NC_CACHE = {}


def get_nc(T, C):
    key = (T, C)
    if key not in _NC_CACHE:
        _NC_CACHE[key] = build_nc(T=T, C=C, HL=C // HS // 2)
    return _NC_CACHE[key]


def kernel(x, w_attn, b_attn, w_proj, b_proj):
    x = np.asarray(x, dtype=np.float32)
    w_attn = np.asarray(w_attn, dtype=np.float32)
    b_attn = np.asarray(b_attn, dtype=np.float32)
    w_proj = np.asarray(w_proj, dtype=np.float32)
    b_proj = np.asarray(b_proj, dtype=np.float32)

    Bq, T, C = x.shape
    nc = get_nc(T, C)

    in_maps = shard_inputs(x, w_attn, b_attn, w_proj, b_proj)
    trace = os.environ.get("KERNEL_TRACE", "0") == "1"
    res = bass_utils.run_bass_kernel_spmd(
        nc, in_maps, core_ids=list(range(N_CORES)), trace=trace)
    if trace and res.exec_time_ns is not None:
        print(f"HW exec time: {res.exec_time_ns} ns", flush=True)
        kernel.last_exec_time_ns = res.exec_time_ns
        kernel.last_results = res
    return gather_outputs(res.results, Bq, T, C)
